# revision 1
# baseline (speedup 1.0000x reference)
"""Trainium2 Bass kernel for nn_ArmaNet02 (ARMA GNN, N=100K, E=1.6M, K=3, T=4, H=16).

Strategy (8 NeuronCores, SPMD):
- dst-sharded: core c owns 12544 (12500 + pad) destination nodes, relabeled by
  in-degree (descending) for tight ELL padding.
- Node state lives in a replicated DRAM table [100352, 64] bf16 (48 conv1
  features + col 48 = narrow state), AllGathered between propagation sweeps.
- Each sweep pulls messages with 4-queue SWDGE dma_gather over QUAD elements
  (4 nodes x 64 cols = 512B, so int16 indices cover all 25088 quads), then does
  a masked multiply + segment-reduce on the Vector engine (mask encodes both
  the edge weight and the quad sub-slot of the source).
- gcn_norm is folded node-wise: table rows are pre-scaled by dis=deg^-1/2 and
  the reduce output is re-scaled by dis[dst], so per-edge values are just the
  raw edge weights (host-permuted into the ELL layout).
- conv1: 1 narrow sweep (A@x) + 3 wide sweeps (48 features); the per-stack
  16x16 weight is applied after the reduce (A commutes with feature maps).
- conv2 has no activation => collapsed to a width-1 Horner chain
  z = v0 + A(v1 + A(v2 + A(v3 + A v4))), 4 narrow sweeps.
"""

import os
import sys
import types
import contextlib
import ctypes

import numpy as np
from ml_dtypes import bfloat16

# ----------------------------------------------------------------------------
# problem constants (hardcoded; kernel.py must be self-contained)
N = 100000
E = 1600000
K = 3
T = 4
H = 16
BN_EPS = 1e-5
NCORE = 8
SHARD = 12500
TPC = 98                 # tiles per core (12544 rows)
ROWS = TPC * 128         # 12544
NT = NCORE * ROWS        # 100352 table rows
NQ = NT // 4             # 25088 quads
CALL = 1024              # slots per dma_gather call (8 columns)
CPC = CALL // 128        # columns per gather call

_EXEC_NS = [None]


def _install_hookshim():
    if "antenv.axon_hooks" in sys.modules:
        return
    try:
        import antenv
    except ImportError:
        return
    mod = types.ModuleType("antenv.axon_hooks")
    mod._hook = None
    mod.set_axon_ntff_profile_hook = lambda h: setattr(mod, "_hook", h)
    mod.get_axon_ntff_profile_hook = lambda: mod._hook
    sys.modules["antenv.axon_hooks"] = mod
    antenv.axon_hooks = mod
    try:
        from trn_agent_boot.trn_boot import _ntff_profile_via_ctypes
        hook = _ntff_profile_via_ctypes("/opt/axon/libaxon_pjrt.so")
        if hook is not None:
            mod.set_axon_ntff_profile_hook(hook)
    except Exception:
        pass


# ----------------------------------------------------------------------------
def _build_layout(counts_all):
    """counts_all: [8, 12500] in-degree per core. Returns shared layout."""
    order = [np.argsort(-counts_all[c], kind="stable") for c in range(NCORE)]
    rank = []
    for c in range(NCORE):
        r = np.empty(SHARD, np.int64)
        r[order[c]] = np.arange(SHARD)
        rank.append(r)
    # sorted (descending) padded counts; per-tile max = first row of tile
    D = np.zeros(TPC, np.int64)
    for c in range(NCORE):
        cs = counts_all[c][order[c]]
        for t in range(TPC):
            lo = t * 128
            if lo < SHARD:
                D[t] = max(D[t], cs[lo])
    # segments: greedy pack whole tiles, cols padded to xCPC
    segc_cap = max(56, int(np.ceil(max(D.max(), 1) / CPC) * CPC))
    segs = []  # list of dict(tiles=[(tile, off, D)], ncols, col0(global), call0)
    cur, curc = [], 0
    for t in range(TPC):
        d = int(D[t])
        if d == 0:
            continue
        if curc + d > segc_cap and cur:
            segs.append((cur, curc))
            cur, curc = [], 0
        cur.append((t, curc, d))
        curc += d
    if cur:
        segs.append((cur, curc))
    out = []
    col0 = 0
    call0 = 0
    for tiles, ncols_raw in segs:
        ncols = int(np.ceil(ncols_raw / CPC) * CPC)
        out.append(dict(tiles=tiles, ncols=ncols, col0=col0, call0=call0,
                        ncalls=ncols // CPC))
        col0 += ncols
        call0 += ncols // CPC
    layout = dict(order=order, rank=rank, D=D, segs=out, ncols=col0,
                  ncalls=call0)
    return layout


def _host_prep(x, edge_index, edge_weight, layout, coef_np, wrow_np):
    """Build per-core device inputs."""
    src = edge_index[0].astype(np.int64)
    dst = edge_index[1].astype(np.int64)
    ew = edge_weight.astype(np.float32)
    order, rank = layout["order"], layout["rank"]
    segs, NCOLS, NCALLS = layout["segs"], layout["ncols"], layout["ncalls"]

    # global relabel: G[orig] = owner*ROWS + rank_owner[local]
    G = np.empty(N, np.int64)
    for c in range(NCORE):
        G[c * SHARD:(c + 1) * SHARD] = c * ROWS + rank[c]

    # per-tile global column base
    colbase = np.full(TPC, -1, np.int64)
    for s in segs:
        for (t, off, d) in s["tiles"]:
            colbase[t] = s["col0"] + off

    owner = dst // SHARD
    gsrc_all = G[src]

    in_maps = []
    for c in range(NCORE):
        m = owner == c
        ls = dst[m] - c * SHARD
        sg = gsrc_all[m]
        ws = ew[m]
        gr = rank[c][ls]                       # 0..12499 sorted rank
        oe = np.argsort(gr, kind="stable")
        gr = gr[oe]; sg = sg[oe]; ws = ws[oe]
        bc = np.bincount(gr, minlength=ROWS)
        starts = np.concatenate([[0], np.cumsum(bc)[:-1]])
        d_within = np.arange(len(gr)) - np.repeat(starts, bc)
        tl = gr // 128
        p = gr % 128
        col = colbase[tl] + d_within
        qidxcol = np.zeros((NCOLS, 128), np.int16)
        wq = np.zeros((128, NCOLS, 4), np.float32)
        qidxcol[col, p] = (sg // 4).astype(np.int16)
        wq[p, col, sg % 4] = ws
        # wrap indices per CALL-slot call
        qc = qidxcol.reshape(NCALLS, CPC * 128)        # I[l] for each call
        w16 = qc.reshape(NCALLS, CALL // 16, 16).transpose(0, 2, 1)
        qidx = np.tile(w16, (1, 8, 1)).transpose(1, 0, 2).reshape(
            128, NCALLS * (CALL // 16))

        xs = np.zeros((128, TPC), np.float32)
        rm = np.zeros((128, TPC), np.float32)
        xv = x[c * SHARD:(c + 1) * SHARD, 0]
        rr = np.arange(SHARD)
        xs[rank[c] % 128, rank[c] // 128] = xv
        rm[rr % 128, rr // 128] = 0.0
        rm2 = np.zeros(ROWS, np.float32); rm2[:SHARD] = 1.0
        rm = rm2.reshape(TPC, 128).T.copy()

        in_maps.append({
            "xsh": xs,
            "rmask": rm,
            "qidx": np.ascontiguousarray(qidx),
            "wq": wq.astype(bfloat16),
            "coef": coef_np,
            "wrow": wrow_np,
        })
    return in_maps, G


# ----------------------------------------------------------------------------
def _build_bass(layout):
    import concourse.bass as bass
    import concourse.mybir as mybir
    import concourse.tile as tile
    from concourse import bacc

    F32 = mybir.dt.float32
    BF16 = mybir.dt.bfloat16
    I16 = mybir.dt.int16
    AO = mybir.AluOpType
    AF = mybir.ActivationFunctionType
    AX = mybir.AxisListType

    segs, NCOLS, NCALLS = layout["segs"], layout["ncols"], layout["ncalls"]
    RG = [list(range(NCORE))]

    nc = bacc.Bacc("TRN2", target_bir_lowering=False, debug=False,
                   num_devices=NCORE, num_swdge_queues=4)

    IW = CALL // 16   # idx words per call per partition
    xsh_d = nc.dram_tensor("xsh", [128, TPC], F32, kind="ExternalInput").ap()
    rmask_d = nc.dram_tensor("rmask", [128, TPC], F32, kind="ExternalInput").ap()
    qidx_d = nc.dram_tensor("qidx", [128, NCALLS * IW], I16, kind="ExternalInput").ap()
    wq_d = nc.dram_tensor("wq", [128, NCOLS, 4], BF16, kind="ExternalInput").ap()
    coef_d = nc.dram_tensor("coef", [128, 320], F32, kind="ExternalInput").ap()
    wrow_d = nc.dram_tensor("wrow", [128, 768], F32, kind="ExternalInput").ap()
    out_d = nc.dram_tensor("out", [128, TPC], F32, kind="ExternalOutput").ap()

    qrr = [0]

    def next_q():
        q = qrr[0] % 4
        qrr[0] += 1
        return q

    with tile.TileContext(nc) as tc:
        with (
            tc.tile_pool(name="pers", bufs=1) as pp,
            tc.tile_pool(name="qseg", bufs=2) as qp,
            tc.tile_pool(name="stage", bufs=3) as sp,
            tc.tile_pool(name="tmpw", bufs=2) as tp,
            tc.tile_pool(name="tmpe", bufs=1) as tpe,
            tc.tile_pool(name="psum", bufs=1, space="PSUM") as psp,
            tc.tile_pool(name="dram", bufs=1, space="DRAM") as dp,
        ):
            # persistent tiles
            xsh = pp.tile([128, TPC], F32)
            rmask = pp.tile([128, TPC], F32)
            wq = pp.tile([128, NCOLS, 4], BF16)
            coef = pp.tile([128, 320], F32)
            wrowT = pp.tile([128, 768], F32)
            dis = pp.tile([128, TPC], F32)
            X = pp.tile([128, TPC, 48], BF16)
            P = pp.tile([128, TPC, 48], F32)
            Z = pp.tile([128, TPC, 48], F32)
            h = pp.tile([128, TPC, 16], BF16)
            hf = pp.tile([128, TPC, 16], F32)
            tw = pp.tile([128, TPC, 64], BF16)
            nar = pp.tile([128, TPC], F32)
            acc = pp.tile([128, TPC], F32)
            tnar = pp.tile([128, TPC], F32)
            vt = pp.tile([128, TPC, 5], F32)
            sums = pp.tile([128, 32], F32)
            bnst = pp.tile([128, 32], F32)
            s16a = pp.tile([128, 16], F32)
            s16b = pp.tile([128, 16], F32)
            s16c = pp.tile([128, 16], F32)
            degt = pp.tile([128, TPC], F32)

            tabA = dp.tile([NT, 64], BF16)
            tabB = dp.tile([NT, 64], BF16)
            tin = dp.tile([ROWS, 64], BF16)
            bnb1 = dp.tile([1, 32], F32)
            bnb2 = dp.tile([1, 32], F32)

            def cap(i):  # coef scalar AP [128,1]
                return coef[:, i:i + 1]

            # loads
            nc.sync.dma_start(xsh[:], xsh_d[:])
            nc.sync.dma_start(rmask[:], rmask_d[:])
            nc.sync.dma_start(wq[:], wq_d[:])
            nc.sync.dma_start(coef[:], coef_d[:])
            nc.sync.dma_start(wrowT[:], wrow_d[:])

            nc.vector.memset(P[:], 0.0)
            nc.vector.memset(nar[:], 0.0)
            nc.vector.memset(tw[:], 0.0)
            nc.vector.memset(degt[:], 0.0)

            # ---------------- P0: degree / dis / initial table ------------
            for s in segs:
                for (t, off, d) in s["tiles"]:
                    c0 = s["col0"] + off
                    nc.vector.tensor_reduce(
                        degt[:, t:t + 1], wq[:, c0:c0 + d, :],
                        axis=AX.XY, op=AO.add)
            t1 = pp.tile([128, TPC], F32)
            nc.vector.scalar_tensor_tensor(t1[:], degt[:], 1e-30, degt[:],
                                           AO.add, AO.max)
            rr1 = pp.tile([128, TPC], F32)
            nc.vector.reciprocal(rr1[:], t1[:])
            ss1 = pp.tile([128, TPC], F32)
            nc.scalar.activation(ss1[:], rr1[:], AF.Sqrt)
            nc.vector.scalar_tensor_tensor(dis[:], degt[:], 0.0, ss1[:],
                                           AO.is_gt, AO.mult)

            def table_write_narrow(src_ap):
                # tw[:, :, 48] = src * dis
                nc.vector.tensor_tensor(
                    tw[:, :, 48:49],
                    src_ap.rearrange("p (t u) -> p t u", u=1),
                    dis[:].rearrange("p (t u) -> p t u", u=1),
                    AO.mult)

            def table_flush(tab):
                nc.sync.dma_start(
                    tin[:].rearrange("(t p) c -> p t c", p=128), tw[:])
                nc.gpsimd.collective_compute(
                    "AllGather", AO.bypass, ins=[tin[:].opt()],
                    outs=[tab[:].opt()], replica_groups=RG)

            table_write_narrow(xsh[:])
            table_flush(tabA)

            # ---------------- gather sweep machinery ----------------------
            def sweep(tab, wide):
                tabq = tab[:].rearrange("(q f) c -> q (f c)", f=4)
                for s in segs:
                    ncalls = s["ncalls"]
                    qs = qp.tile([128, ncalls * IW], I16, tag="qs")
                    nc.sync.dma_start(
                        qs[:], qidx_d[:, s["call0"] * IW:(s["call0"] + ncalls) * IW])
                    st = sp.tile([128, s["ncols"], 256], BF16, tag="st")
                    for ci in range(ncalls):
                        nc.gpsimd.dma_gather(
                            st[:, CPC * ci:CPC * (ci + 1), :], tabq,
                            qs[:, ci * IW:(ci + 1) * IW],
                            CALL, CALL, 256, single_packet=True,
                            queue_num=next_q())
                    for (t, off, d) in s["tiles"]:
                        c0 = s["col0"] + off
                        if wide:
                            sv = st[:, off:off + d, :].rearrange(
                                "p d (q j) -> p d q j", q=4)[:, :, :, 0:48]
                            wv = wq[:, c0:c0 + d, :].rearrange(
                                "p d (q u) -> p d q u", u=1).to_broadcast(
                                [128, d, 4, 48])
                            nc.vector.tensor_tensor(sv, sv, wv, AO.mult)
                            rv = st[:, off:off + d, :].rearrange(
                                "p d (q j) -> p j d q", q=4)[:, 0:48]
                            nc.vector.tensor_reduce(
                                P[:, t, :], rv, axis=AX.XY, op=AO.add)
                        else:
                            tmpn = tp.tile([128, d, 4], BF16, tag="tmpn")
                            mv = st[:, off:off + d, :].rearrange(
                                "p d (q j) -> p d q j", q=4)[:, :, :, 48]
                            nc.vector.tensor_tensor(
                                tmpn[:], mv, wq[:, c0:c0 + d, :], AO.mult)
                            nc.vector.tensor_reduce(
                                nar[:, t:t + 1], tmpn[:], axis=AX.XY, op=AO.add)

            def build_root_into_Z():
                # Z[:, :, c] = x * rootw_c + b_c
                for c in range(48):
                    bb = coef[:, 96 + c:97 + c].rearrange(
                        "p (t u) -> p t u", u=1).to_broadcast([128, TPC, 1])
                    nc.vector.scalar_tensor_tensor(
                        Z[:, :, c:c + 1],
                        xsh[:].rearrange("p (t u) -> p t u", u=1),
                        cap(48 + c), bb, AO.mult, AO.add)

            TG = 21  # einsum tile-group

            def conv1_combine():
                # P *= dis ; Z = root + P@Wblk ; X = relu(Z)
                nc.vector.tensor_tensor(
                    P[:], P[:],
                    dis[:].rearrange("p (t u) -> p t u", u=1).to_broadcast(
                        [128, TPC, 48]), AO.mult)
                build_root_into_Z()
                for g0 in range(0, TPC, TG):
                    tg = min(TG, TPC - g0)
                    for k in range(K):
                        tmpE = tpe.tile([128, TG, 16, 16], BF16, tag="tmpE")
                        pb = P[:, g0:g0 + tg, k * 16:(k + 1) * 16].rearrange(
                            "p t (u i) -> p t u i", u=1).to_broadcast(
                            [128, tg, 16, 16])
                        wv = wrowT[:, k * 256:(k + 1) * 256].rearrange(
                            "p (u o i) -> p u o i", u=1, o=16).to_broadcast(
                            [128, tg, 16, 16])
                        nc.vector.tensor_tensor(tmpE[:, 0:tg], pb, wv, AO.mult)
                        zt = tpe.tile([128, TG, 16], F32, tag="ztE")
                        nc.vector.tensor_reduce(zt[:, 0:tg], tmpE[:, 0:tg],
                                                axis=AX.X, op=AO.add)
                        nc.vector.tensor_tensor(
                            Z[:, g0:g0 + tg, k * 16:(k + 1) * 16],
                            Z[:, g0:g0 + tg, k * 16:(k + 1) * 16],
                            zt[:, 0:tg], AO.add)
                nc.scalar.activation(X[:], Z[:], AF.Relu)

            def wide_post(tab_next):
                conv1_combine()
                nc.vector.tensor_tensor(
                    tw[:, :, 0:48], X[:],
                    dis[:].rearrange("p (t u) -> p t u", u=1).to_broadcast(
                        [128, TPC, 48]), AO.mult)
                table_flush(tab_next)

            # ---------------- P1: x-sweep -> out1 --------------------------
            sweep(tabA, wide=False)
            axv = pp.tile([128, TPC], F32)
            nc.vector.tensor_tensor(axv[:], nar[:], dis[:], AO.mult)
            build_root_into_Z()
            for c in range(48):
                nc.vector.scalar_tensor_tensor(
                    Z[:, :, c:c + 1],
                    axv[:].rearrange("p (t u) -> p t u", u=1),
                    cap(0 + c), Z[:, :, c:c + 1], AO.mult, AO.add)
            nc.scalar.activation(X[:], Z[:], AF.Relu)
            nc.vector.tensor_tensor(
                tw[:, :, 0:48], X[:],
                dis[:].rearrange("p (t u) -> p t u", u=1).to_broadcast(
                    [128, TPC, 48]), AO.mult)
            table_flush(tabB)

            # ---------------- P2..P4 wide sweeps ---------------------------
            sweep(tabB, wide=True)
            wide_post(tabA)
            sweep(tabA, wide=True)
            wide_post(tabB)
            sweep(tabB, wide=True)
            # last conv1 iteration: produce h (no table flush of X needed)
            conv1_combine()

            # h1 = mean over stacks
            nc.vector.tensor_tensor(hf[:], X[:, :, 0:16], X[:, :, 16:32], AO.add)
            nc.vector.tensor_tensor(hf[:], hf[:], X[:, :, 32:48], AO.add)
            nc.scalar.activation(hf[:], hf[:], AF.Copy, scale=1.0 / 3.0)

            # ---------------- BatchNorm -----------------------------------
            nc.vector.tensor_tensor(
                hf[:], hf[:],
                rmask[:].rearrange("p (t u) -> p t u", u=1).to_broadcast(
                    [128, TPC, 16]), AO.mult)
            nc.vector.tensor_reduce(
                sums[:, 0:16], hf[:].rearrange("p t f -> p f t"),
                axis=AX.X, op=AO.add)
            hsq = pp.tile([128, TPC, 16], F32)
            nc.scalar.activation(hsq[:], hf[:], AF.Square)
            nc.vector.tensor_reduce(
                sums[:, 16:32], hsq[:].rearrange("p t f -> p f t"),
                axis=AX.X, op=AO.add)
            ones_ps = psp.tile([1, 32], F32)
            nc.tensor.matmul(ones_ps[:], coef[:, 263:264], sums[:],
                             start=True, stop=True)
            bnl = pp.tile([1, 32], F32)
            nc.scalar.activation(bnl[:], ones_ps[:], AF.Copy)
            nc.sync.dma_start(bnb1[:], bnl[:])
            nc.gpsimd.collective_compute(
                "AllReduce", AO.add, ins=[bnb1[:].opt()], outs=[bnb2[:].opt()],
                replica_groups=RG)
            nc.sync.dma_start(bnst[:], bnb2[:].to_broadcast([128, 32]))
            nc.scalar.activation(s16a[:], bnst[:, 0:16], AF.Copy, scale=1.0 / N)  # mean
            nc.scalar.activation(s16b[:], bnst[:, 16:32], AF.Copy, scale=1.0 / N)  # E[x^2]
            musq = pp.tile([128, 16], F32)
            nc.scalar.activation(musq[:], s16a[:], AF.Square)
            nc.vector.tensor_tensor(s16b[:], s16b[:], musq[:], AO.subtract)  # var
            sd = pp.tile([128, 16], F32)
            nc.vector.scalar_tensor_tensor(sd[:], s16b[:], BN_EPS, s16b[:],
                                           AO.add, AO.max)
            nc.scalar.activation(sd[:], sd[:], AF.Sqrt)
            rsd = pp.tile([128, 16], F32)
            nc.vector.reciprocal(rsd[:], sd[:])
            nc.vector.tensor_tensor(s16c[:], rsd[:], coef[:, 144:160], AO.mult)  # scale
            shf = pp.tile([128, 16], F32)
            nc.vector.tensor_tensor(shf[:], s16a[:], s16c[:], AO.mult)
            nc.vector.tensor_tensor(shf[:], coef[:, 160:176], shf[:], AO.subtract)
            nc.vector.tensor_tensor(
                hf[:], hf[:],
                s16c[:].rearrange("p (u f) -> p u f", u=1).to_broadcast(
                    [128, TPC, 16]), AO.mult)
            nc.vector.tensor_tensor(
                hf[:], hf[:],
                shf[:].rearrange("p (u f) -> p u f", u=1).to_broadcast(
                    [128, TPC, 16]), AO.add)
            nc.scalar.activation(h[:], hf[:], AF.Relu)

            # ---------------- conv2 projections ----------------------------
            for j in range(5):
                bb = coef[:, 176 + j:177 + j].rearrange(
                    "p (t u) -> p t u", u=1).to_broadcast([128, TPC, 1])
                nc.vector.scalar_tensor_tensor(
                    vt[:, :, j:j + 1], h[:, :, 0:1],
                    cap(181 + j * 16 + 0), bb, AO.mult, AO.add)
                for i in range(1, 16):
                    nc.vector.scalar_tensor_tensor(
                        vt[:, :, j:j + 1], h[:, :, i:i + 1],
                        cap(181 + j * 16 + i), vt[:, :, j:j + 1],
                        AO.mult, AO.add)

            # ---------------- Horner chain ----------------------------------
            nc.scalar.activation(
                acc[:], vt[:, :, 4:5].rearrange("p t u -> p (t u)"), AF.Copy)
            tabs = [tabA, tabB]
            for step, j in enumerate((3, 2, 1, 0)):
                table_write_narrow(acc[:])
                tab = tabs[step % 2]
                table_flush(tab)
                sweep(tab, wide=False)
                nc.vector.tensor_tensor(tnar[:], nar[:], dis[:], AO.mult)
                nc.vector.tensor_tensor(
                    acc[:], tnar[:],
                    vt[:, :, j:j + 1].rearrange("p t u -> p (t u)"), AO.add)

            # ---------------- final linear + sigmoid ------------------------
            outb = pp.tile([128, TPC], F32)
            nc.scalar.activation(outb[:], acc[:], AF.Sigmoid,
                                 scale=cap(261), bias=cap(262))
            nc.sync.dma_start(out_d[:], outb[:])

    nc.compile()
    return nc


# ----------------------------------------------------------------------------
def kernel(x, edge_index, edge_weight, w1_init, w1_w, w1_root, w1_b,
           bn1_g, bn1_b, w2_init, w2_w, w2_root, w2_b, lin_w, lin_b):
    _install_hookshim()
    x = np.asarray(x, np.float32)
    edge_index = np.asarray(edge_index)
    edge_weight = np.asarray(edge_weight, np.float32)

    counts_all = np.zeros((NCORE, SHARD), np.int64)
    dst = edge_index[1].astype(np.int64)
    for c in range(NCORE):
        m = (dst // SHARD) == c
        counts_all[c] = np.bincount(dst[m] - c * SHARD, minlength=SHARD)
    layout = _build_layout(counts_all)

    # ---- coefficient packing (host): tiny-weight derived scalars
    w1_init = np.asarray(w1_init, np.float64)
    w1_w_ = np.asarray(w1_w, np.float64)
    w1_root = np.asarray(w1_root, np.float64)
    w1_b_ = np.asarray(w1_b, np.float64)
    w2_init = np.asarray(w2_init, np.float64)
    w2_w_ = np.asarray(w2_w, np.float64)
    w2_root = np.asarray(w2_root, np.float64)
    w2_b_ = np.asarray(w2_b, np.float64)

    coef = np.zeros(320, np.float64)
    coef[0:48] = w1_init[:, 0, :].reshape(-1)
    coef[48:96] = w1_root[:, 0, :].reshape(-1)
    coef[96:144] = w1_b_.reshape(-1)
    coef[144:160] = np.asarray(bn1_g, np.float64)
    coef[160:176] = np.asarray(bn1_b, np.float64)
    wk = w2_w_[:, 0, 0]
    gmat = np.zeros((5, 16), np.float64)
    beta = np.zeros(5, np.float64)
    gmat[4] = (wk ** 3 / 3.0) @ w2_init[:, :, 0]
    gmat[3] = (wk ** 3 / 3.0) @ w2_root[:, :, 0]; beta[3] = (wk ** 3 / 3.0) @ w2_b_[:, 0]
    gmat[2] = (wk ** 2 / 3.0) @ w2_root[:, :, 0]; beta[2] = (wk ** 2 / 3.0) @ w2_b_[:, 0]
    gmat[1] = (wk / 3.0) @ w2_root[:, :, 0];      beta[1] = (wk / 3.0) @ w2_b_[:, 0]
    gmat[0] = np.ones(3) / 3.0 @ w2_root[:, :, 0]; beta[0] = np.ones(3) / 3.0 @ w2_b_[:, 0]
    coef[176:181] = beta
    coef[181:261] = gmat.reshape(-1)
    coef[261] = np.asarray(lin_w, np.float64)[0, 0]
    coef[262] = np.asarray(lin_b, np.float64)[0]
    coef[263] = 1.0
    coef_np = np.tile(coef.astype(np.float32)[None, :], (128, 1))

    wrow = np.zeros(768, np.float64)
    for k in range(K):
        for o in range(16):
            wrow[(k * 16 + o) * 16:(k * 16 + o) * 16 + 16] = w1_w_[k, :, o]
    wrow_np = np.tile(wrow.astype(np.float32)[None, :], (128, 1))

    in_maps, G = _host_prep(x, edge_index, edge_weight, layout, coef_np, wrow_np)

    nc = _build_bass(layout)
    from concourse.bass_utils import run_bass_kernel_spmd
    trace = os.environ.get("BASS_GNN_TRACE", "0") == "1"
    res = run_bass_kernel_spmd(nc, in_maps, core_ids=list(range(NCORE)),
                               trace=trace)
    _EXEC_NS[0] = res.exec_time_ns

    out = np.empty((N, 1), np.float32)
    for c in range(NCORE):
        ob = res.results[c]["out"]        # [128, TPC]
        r = np.arange(SHARD)
        vals = ob[r % 128, r // 128]       # value at rank r
        out[c * SHARD + layout["order"][c], 0] = vals
    return out


def last_exec_ns():
    return _EXEC_NS[0]

